# revision 29
# baseline (speedup 1.0000x reference)
"""Trainium2 kernel for nn_BinaryTokenClassificationModel.

Math (per batch sample b):
    src = seq[src_idx]           # (K, H) gather of K masked rows
    tgt = seq[tgt_idx]           # (K, H)
    col[s] = src[s] @ w[:H] + tgt[s] @ w[H:] + bias
    out[s, t] = col[s]           # broadcast over t

Sharding: data-parallel over batch B=8 across 8 NeuronCores (one sample per
core). Masks are converted to gather indices on the host (cheap O(B*L) argsort
metadata prep, matching the reference's stable-argsort semantics); the heavy
data stays on device.

Per core the kernel touches only the 2*K masked rows (4 MiB of the 16 MiB
sample) via SWDGE indirect-DMA row gathers plus the 1 MiB output write — the
memory roofline for this problem (~5.25 MiB of HBM traffic at ~360 GB/s/core):

  - weights+bias are loaded once as a [1, 2H+1] 8 KiB strip and replicated
    across the 128 SBUF partitions with an SBUF->SBUF HWDGE DMA whose source
    AP has a 0-stride repeat dim — no PE/PSUM round trip and, unlike a
    host-replicated (P, 2H) tensor, no extra 1 MiB of HBM reads (the sim's
    single-FIFO DMA device charges it like HBM traffic, but on HW it rides
    the SBUF fabric (435 GB/s) inside the SDMA engines' headroom while the
    HBM pipe (~358 GB/s) stays gather-bound). The strip is laid out
    [bias | w_src | w_tgt] and broadcast in two chunks so the first
    mul-reduce only waits for the first chunk.
  - all 9 row gathers (4 tiles x {src,tgt} into one (128, 2H) tile per tile;
    the last tile's tgt split 3/4+1/4) are issued up-front right after the
    tiny index load, so the SWDGE descriptor ring keeps the SDMA engines
    saturated end-to-end. src/tgt stay SEPARATE DMAs: the src mul-reduce
    overlaps the tgt transfer (a merged 2-row gather stalls each tile's
    compute for the doubled transfer), which is worth more than the ~1 us
    fixed Q7 descriptor-generation cost the extra instruction costs.
  - per 128-token tile: ONE fused DVE scalar_tensor_tensor per mask computes
    the elementwise product AND its free-dim sum (accum_out) in a single 1x
    pass; a tiny STT folds the bias into col = (ds + b) + dt; a 2x
    tensor_scalar broadcasts col over t; HWDGE writes the (128, K) output
    slab, alternating the SP/ACT rings per tile, with stores issued between
    late gathers to keep them off the kernel tail.
  - tail: the last tile's tgt gather is split 768+256 elements so only a
    ~330 ns reduce remains after the final gather lands, and the last store
    is split across both HWDGE rings.
  - drain: the Tile-exit sem waits and sem resets are distributed across all
    five engines so the epilogue resolves in parallel.

Container quirks handled by _patch_tile_drain(): this walrus accepts at most
one sync wait per instruction (extra waits are split onto single-wait NOPs)
and cannot ingest EVENT_SEMAPHORE_RANGE_CLEAR (semaphores are reset via
per-sem sem-wr-imm NOP updates instead).
"""

import math

import numpy as np

P = 128  # SBUF partitions

_PATCHED = False


def _patch_tile_drain():
    """This container's walrus/bass accept only ONE sync wait per instruction,
    but Tile attaches one wait per outstanding dependency to a single
    instruction ("Too many sync wait commands"). Split the extra waits across
    dedicated single-wait NOPs inserted just before on the same engine (the
    engine executes in order, so waiting sequentially is equivalent)."""
    global _PATCHED
    if _PATCHED:
        return
    import concourse.mybir as mybir
    import concourse.tile as tile_mod
    from concourse.vector_clock import ScopedClock

    _orig_add = tile_mod.TileContext._add_instruction

    def _add_instruction(self, inst):
        si = inst.sync_info
        if si is not None and si.on_wait and len(si.on_wait) > 1:
            waits = list(si.on_wait)
            si.on_wait = waits[-1:]
            for j, w in enumerate(waits[:-1]):
                nop = mybir.InstNoOp(name=f"{inst.name}_ws{j}")
                nop.engine = inst.engine
                nop.sync_info = mybir.SyncInfo(on_wait=[w], on_update=[])
                _orig_add(self, nop)
        _orig_add(self, inst)

    def _drain_and_barrier(self, tick_clock, wait_clock):
        nc = self.nc
        drain_bi = nc.sync.drain()
        wait_clock.add_sem_waits(
            drain_bi.ins, ScopedClock({None: tick_clock.global_clock})
        )
        si = drain_bi.ins.sync_info
        if si is not None and si.on_wait and len(si.on_wait) > 1:
            # Distribute the extra waits across all engines' single-wait NOPs
            # so they resolve in parallel (this walrus allows only one wait
            # per instruction); the following all_engine_barrier makes the
            # union globally visible.
            waits = list(si.on_wait)
            si.on_wait = waits[:1]
            wait_engines = [nc.scalar, nc.gpsimd, nc.vector, nc.tensor, nc.sync]
            for j, w in enumerate(waits[1:]):
                nop_bi = wait_engines[j % len(wait_engines)].nop(
                    nofuse=True, hint="wait_split"
                )
                nop_bi.ins.sync_info = mybir.SyncInfo(on_wait=[w], on_update=[])

        nc.all_engine_barrier()
        assert self.sems is not None
        popped = nc._tile_sem_poison_stack.pop()
        assert popped is self._sem_poison
        # Inline clear_and_free_semaphores, but reset the sems with batched
        # multi-update NOPs (sem-wr-imm 0) instead of the
        # EVENT_SEMAPHORE_RANGE_CLEAR InstISA this walrus can't ingest
        # ("ISA wrong length"). Only *waits* are limited to one per
        # instruction; updates can be batched.
        from concourse.bass import compact_to_ranges

        sems = list(self.sems.allocated().values())
        if sems:
            sem_nums = [s.num for s in sems]
            for sem_range in compact_to_ranges(sem_nums):
                assert nc._state.free_isdisjoint(sem_range)
                nc.gpsimd.dma_reset(sem_range)
            reset_engines = [nc.gpsimd, nc.scalar, nc.vector, nc.tensor, nc.sync]
            for j, s in enumerate(sems):
                nop_bi = reset_engines[j % len(reset_engines)].nop(
                    nofuse=True, hint="sem_reset"
                )
                nop_bi.ins.sync_info = mybir.SyncInfo(
                    on_wait=[],
                    on_update=[
                        mybir.SyncUpdate(
                            sync_type="semaphore",
                            id=s.num,
                            ant_name=s.name,
                            update_mode="sem-wr-imm",
                            update_value=0,
                        )
                    ],
                )
            nc._state.prepend_free_semaphores(sem_nums)
            for poison_set in nc._tile_sem_poison_stack:
                poison_set.update(sem_nums)
        nc.all_engine_barrier()

    tile_mod.TileContext._add_instruction = _add_instruction
    tile_mod.TileContext._drain_and_barrier = _drain_and_barrier
    _PATCHED = True


def _build_nc(L, H, K, n_tiles, repeat=1, w_mode="dma"):
    import concourse.bass as bass
    import concourse.mybir as mybir
    import concourse.tile as tile

    _patch_tile_drain()

    f32 = mybir.dt.float32
    i32 = mybir.dt.int32

    nc = bass.Bass("TRN2")
    seq = nc.dram_tensor("seq", [L, H], f32, kind="ExternalInput")
    # idx[p, 2*t] = src index for token t*P+p; idx[p, 2*t+1] = tgt index
    idx = nc.dram_tensor("idx", [P, 2 * n_tiles], i32, kind="ExternalInput")
    # [bias | w_src (H) | w_tgt (H)] as a single replicable strip
    wsb = nc.dram_tensor("wsb", [1, 2 * H + 1], f32, kind="ExternalInput")
    if w_mode == "wcat":
        wcat = nc.dram_tensor("wcat", [P, 2 * H + 1], f32, kind="ExternalInput")
    out = nc.dram_tensor("out", [K, K], f32, kind="ExternalOutput")

    last_t = n_tiles - 1
    Hs = (3 * H) // 4  # uneven split of the last tgt gather: 3/4 then 1/4
    mult = mybir.AluOpType.mult
    add = mybir.AluOpType.add

    with tile.TileContext(nc) as tc:
        with (
            tc.tile_pool(name="cpool", bufs=1) as cpool,
            tc.tile_pool(name="wpool", bufs=2) as wpool,
        ):
            idx_t = cpool.tile([P, 2 * n_tiles], i32)
            nc.sync.dma_start(out=idx_t[:], in_=idx[:])
            ws = cpool.tile([1, 2 * H + 1], f32)
            if w_mode == "dma":
                nc.scalar.dma_start(out=ws[:], in_=wsb[:])

            def gather(dst, idx_ap, element_offset=0):
                nc.gpsimd.indirect_dma_start(
                    out=dst,
                    out_offset=None,
                    in_=seq[:],
                    in_offset=bass.IndirectOffsetOnAxis(ap=idx_ap, axis=0),
                    element_offset=element_offset,
                )

            def mul_reduce(prod_ap, gath_ap, w_ap, d_ap):
                # single DVE op: prod = gath * w, d = sum(prod)
                nc.vector.scalar_tensor_tensor(
                    out=prod_ap,
                    in0=gath_ap,
                    scalar=1.0,
                    in1=w_ap,
                    op0=mult,
                    op1=mult,
                    accum_out=d_ap,
                )

            def one_rep():
                # --- weights: 8 KiB strip + on-chip partition replication ---
                wb = wpool.tile([P, 2 * H + 1], f32, tag="wb")
                if w_mode == "dma":
                    # chunk 1 = bias + w_src (unblocks the first mul_reduce),
                    # chunk 2 = w_tgt; SBUF->SBUF on the ACT HWDGE ring with a
                    # 0-stride repeat dim: no HBM traffic, overlaps gathers.
                    c1 = 1 + H
                    nc.scalar.dma_start(
                        out=wb[:, 0:c1],
                        in_=ws[0:1, 0:c1].unsqueeze(1).broadcast_to((1, P, c1)),
                    )
                    nc.scalar.dma_start(
                        out=wb[:, c1 : 2 * H + 1],
                        in_=ws[0:1, c1 : 2 * H + 1]
                        .unsqueeze(1)
                        .broadcast_to((1, P, H)),
                    )
                else:  # host-replicated fallback: 1 MiB HBM read
                    nc.scalar.dma_start(out=wb[:], in_=wcat[:])
                b_ap = wb[:, 0:1]

                def w_src(rr):
                    return wb[rr, 1 : 1 + H]

                def w_tgt_sl(rr, lo, hi):
                    return wb[rr, 1 + H + lo : 1 + H + hi]

                # --- gathers (issued eagerly, stores interleaved late) ---
                # per tile, src||tgt rows land side by side in one (P, 2H)
                # tile; fetched by separate DMAs (see issue_gathers)
                cst = [
                    wpool.tile([P, 2 * H], f32, tag=f"cst{t}", name=f"cst{t}")
                    for t in range(n_tiles)
                ]

                def issue_gathers(t):
                    # src and tgt rows fetched by separate DMAs: the src
                    # mul-reduce overlaps the tgt transfer, keeping DVE fed
                    # at the per-row cadence (a merged 2-row gather stalls
                    # each tile's compute for the full doubled transfer)
                    rows = min(P, K - t * P)
                    rr = slice(0, rows)
                    if t != last_t:
                        gather(cst[t][rr, 0:H], idx_t[rr, 2 * t : 2 * t + 1])
                        gather(
                            cst[t][rr, H : 2 * H],
                            idx_t[rr, 2 * t + 1 : 2 * t + 2],
                        )
                    else:
                        # tail: src full, tgt split 3/4+1/4 so only a short
                        # reduce remains after the last gather lands
                        gather(cst[t][rr, 0:H], idx_t[rr, 2 * t : 2 * t + 1])
                        idx3 = idx_t[rr, 2 * t + 1 : 2 * t + 2]
                        gather(cst[t][rr, H : H + Hs], idx3)
                        gather(cst[t][rr, H + Hs :], idx3, element_offset=Hs)

                ps = wpool.tile([P, H], f32, tag="ps")  # shared DVE scratch
                obts = []

                def compute_tile(t):
                    rows = min(P, K - t * P)
                    rr = slice(0, rows)
                    ds = wpool.tile([P, 1], f32, tag=f"ds{t}", name=f"ds{t}")
                    dt = wpool.tile([P, 1], f32, tag=f"dt{t}", name=f"dt{t}")
                    col = wpool.tile([P, 1], f32, tag=f"col{t}", name=f"col{t}")
                    obt = wpool.tile([P, K], f32, tag=f"obt{t}", name=f"obt{t}")
                    mul_reduce(ps[rr], cst[t][rr, 0:H], w_src(rr), ds[rr, 0:1])
                    if t != last_t:
                        mul_reduce(
                            ps[rr], cst[t][rr, H : 2 * H], w_tgt_sl(rr, 0, H), dt[rr, 0:1]
                        )
                        # col = (ds + bias) + dt in one DVE op
                        nc.vector.scalar_tensor_tensor(
                            out=col[rr, 0:1],
                            in0=ds[rr, 0:1],
                            scalar=b_ap[rr],
                            in1=dt[rr, 0:1],
                            op0=add,
                            op1=add,
                        )
                    else:
                        dt2 = wpool.tile([P, 1], f32, tag="dt2")
                        col0 = wpool.tile([P, 1], f32, tag="col0")
                        mul_reduce(
                            ps[rr, 0:Hs],
                            cst[t][rr, H : H + Hs],
                            w_tgt_sl(rr, 0, Hs),
                            dt[rr, 0:1],
                        )
                        nc.vector.scalar_tensor_tensor(
                            out=col0[rr, 0:1],
                            in0=ds[rr, 0:1],
                            scalar=b_ap[rr],
                            in1=dt[rr, 0:1],
                            op0=add,
                            op1=add,
                        )
                        mul_reduce(
                            ps[rr, Hs:H],
                            cst[t][rr, H + Hs : 2 * H],
                            w_tgt_sl(rr, Hs, H),
                            dt2[rr, 0:1],
                        )
                        nc.vector.tensor_tensor(
                            out=col[rr, 0:1],
                            in0=col0[rr, 0:1],
                            in1=dt2[rr, 0:1],
                            op=add,
                        )
                    # broadcast col over the K output columns (in0 * 0 + col)
                    nc.vector.tensor_scalar(
                        out=obt[rr],
                        in0=cst[t][rr, 0:K],
                        scalar1=0.0,
                        scalar2=col[rr, 0:1],
                        op0=mult,
                        op1=add,
                    )
                    obts.append((t, rows, obt))

                def issue_store(t, rows, obt):
                    rr = slice(0, rows)
                    if t != last_t:
                        eng = nc.sync if t % 2 == 0 else nc.scalar
                        eng.dma_start(out=out[t * P : t * P + rows, :], in_=obt[rr])
                    else:
                        Kh = K // 2
                        nc.scalar.dma_start(
                            out=out[t * P : t * P + rows, 0:Kh], in_=obt[rr, 0:Kh]
                        )
                        nc.sync.dma_start(
                            out=out[t * P : t * P + rows, Kh:K], in_=obt[rr, Kh:K]
                        )

                # Gathers for the first tiles go out immediately; each later
                # tile's gathers are followed by the store of the tile whose
                # data is ready by then, keeping the DMA stream dense and the
                # final stores off the kernel tail.
                for t in range(min(2, n_tiles)):
                    issue_gathers(t)
                for t in range(n_tiles):
                    compute_tile(t)
                    if t + 2 < n_tiles:
                        issue_gathers(t + 2)
                    if t >= 1:
                        issue_store(*obts[t - 1])
                issue_store(*obts[last_t])

            for _ in range(repeat):
                one_rep()
    return nc


_CACHE = {}

# test.py can flip these to get a profiled run
TRACE = False
LAST_RESULTS = None


def prepare_in_maps(seq, src_mask, tgt_mask, w, b, K):
    """Host-side metadata prep: mask -> ordered gather indices (matches the
    reference's stable argsort semantics exactly) + the [bias|w] strip."""
    B, L, H = seq.shape
    n_tiles = math.ceil(K / P)

    src_idx = np.argsort(~src_mask, axis=1, kind="stable")[:, :K].astype(np.int32)
    tgt_idx = np.argsort(~tgt_mask, axis=1, kind="stable")[:, :K].astype(np.int32)

    # Per-core index layout (P, 2*n_tiles): interleave src/tgt per tile
    idx_host = np.zeros((B, P, 2 * n_tiles), dtype=np.int32)
    for t in range(n_tiles):
        rows = min(P, K - t * P)
        idx_host[:, :rows, 2 * t] = src_idx[:, t * P : t * P + rows]
        idx_host[:, :rows, 2 * t + 1] = tgt_idx[:, t * P : t * P + rows]

    wsb_host = np.concatenate(
        [np.asarray(b[:1], np.float32), np.asarray(w, np.float32)]
    )[None, :]
    wsb_host = np.ascontiguousarray(wsb_host)
    # host-replicated variant, only consumed by w_mode="wcat" builds (A/B)
    wcat_host = np.ascontiguousarray(np.broadcast_to(wsb_host, (P, 2 * H + 1)))
    return [
        {
            "seq": seq[bb],
            "idx": idx_host[bb],
            "wsb": wsb_host,
            "wcat": wcat_host,
        }
        for bb in range(B)
    ]


def kernel(sequence_output, source_mask, target_mask, weight, bias, num_tokens):
    global LAST_RESULTS
    from concourse.bass_utils import run_bass_kernel_spmd

    seq = np.ascontiguousarray(np.asarray(sequence_output, dtype=np.float32))
    src_mask = np.asarray(source_mask, dtype=bool)
    tgt_mask = np.asarray(target_mask, dtype=bool)
    w = np.asarray(weight, dtype=np.float32)
    b = np.asarray(bias, dtype=np.float32)
    K = int(num_tokens)

    B, L, H = seq.shape
    n_tiles = math.ceil(K / P)

    in_maps = prepare_in_maps(seq, src_mask, tgt_mask, w, b, K)

    key = (L, H, K, n_tiles)
    if key not in _CACHE:
        _CACHE[key] = _build_nc(L, H, K, n_tiles)
    nc = _CACHE[key]

    # one sample per core; batches beyond 8 run in chunks of 8 cores
    N_CORES = 8
    outs = []
    for lo in range(0, B, N_CORES):
        chunk = in_maps[lo : lo + N_CORES]
        res = run_bass_kernel_spmd(
            nc, chunk, core_ids=list(range(len(chunk))), trace=TRACE
        )
        LAST_RESULTS = res
        outs.extend(res.results[i]["out"] for i in range(len(chunk)))
    return np.stack(outs, axis=0)


# revision 32
# speedup vs baseline: 1.0578x; 1.0578x over previous
"""Trainium2 kernel for nn_BinaryTokenClassificationModel.

Math (per batch sample b):
    src = seq[src_idx]           # (K, H) gather of K masked rows
    tgt = seq[tgt_idx]           # (K, H)
    col[s] = src[s] @ w[:H] + tgt[s] @ w[H:] + bias
    out[s, t] = col[s]           # broadcast over t

Sharding: data-parallel over batch B=8 across 8 NeuronCores (one sample per
core). Masks are converted to gather indices on the host (cheap O(B*L) argsort
metadata prep, matching the reference's stable-argsort semantics); the heavy
data stays on device.

Per core the kernel touches only the 2*K masked rows (4 MiB of the 16 MiB
sample) via SWDGE indirect-DMA row gathers plus the 1 MiB output write — the
memory roofline for this problem (~5.25 MiB of HBM traffic at ~360 GB/s/core):

  - weights+bias are loaded once as a [1, 2H+1] 8 KiB strip and replicated
    across the 128 SBUF partitions with an SBUF->SBUF HWDGE DMA whose source
    AP has a 0-stride repeat dim — no PE/PSUM round trip and, unlike a
    host-replicated (P, 2H) tensor, no extra 1 MiB of HBM reads (the sim's
    single-FIFO DMA device charges it like HBM traffic, but on HW it rides
    the SBUF fabric (435 GB/s) inside the SDMA engines' headroom while the
    HBM pipe (~358 GB/s) stays gather-bound). The strip is laid out
    [bias | w_src | w_tgt] and broadcast in two chunks so the first
    mul-reduce only waits for the first chunk.
  - all 9 row gathers (4 tiles x {src,tgt} into one (128, 2H) tile per tile;
    the last tile's tgt split 3/4+1/4) are issued up-front right after the
    tiny index load, so the SWDGE descriptor ring keeps the SDMA engines
    saturated end-to-end. src/tgt stay SEPARATE DMAs: the src mul-reduce
    overlaps the tgt transfer (a merged 2-row gather stalls each tile's
    compute for the doubled transfer), which is worth more than the ~1 us
    fixed Q7 descriptor-generation cost the extra instruction costs.
  - per 128-token tile: ONE fused DVE scalar_tensor_tensor per mask computes
    the elementwise product AND its free-dim sum (accum_out) in a single 1x
    pass; a tiny STT folds the bias into col = (ds + b) + dt; a 2x
    tensor_scalar broadcasts col over t; HWDGE writes the (128, K) output
    slab, alternating the SP/ACT rings per tile, with stores issued between
    late gathers to keep them off the kernel tail.
  - tail: the last tile's tgt gather is split 768+256 elements so only a
    ~330 ns reduce remains after the final gather lands, and the last store
    is split across both HWDGE rings.
  - drain: the Tile-exit sem waits and sem resets are distributed across all
    five engines so the epilogue resolves in parallel.

Container quirks handled by _patch_tile_drain(): this walrus accepts at most
one sync wait per instruction (extra waits are split onto single-wait NOPs)
and cannot ingest EVENT_SEMAPHORE_RANGE_CLEAR (semaphores are reset via
per-sem sem-wr-imm NOP updates instead).
"""

import math

import numpy as np

P = 128  # SBUF partitions

_PATCHED = False


def _patch_tile_drain():
    """This container's walrus/bass accept only ONE sync wait per instruction,
    but Tile attaches one wait per outstanding dependency to a single
    instruction ("Too many sync wait commands"). Split the extra waits across
    dedicated single-wait NOPs inserted just before on the same engine (the
    engine executes in order, so waiting sequentially is equivalent)."""
    global _PATCHED
    if _PATCHED:
        return
    import concourse.mybir as mybir
    import concourse.tile as tile_mod
    from concourse.vector_clock import ScopedClock

    _orig_add = tile_mod.TileContext._add_instruction

    def _add_instruction(self, inst):
        si = inst.sync_info
        if si is not None and si.on_wait and len(si.on_wait) > 1:
            waits = list(si.on_wait)
            si.on_wait = waits[-1:]
            for j, w in enumerate(waits[:-1]):
                nop = mybir.InstNoOp(name=f"{inst.name}_ws{j}")
                nop.engine = inst.engine
                nop.sync_info = mybir.SyncInfo(on_wait=[w], on_update=[])
                _orig_add(self, nop)
        _orig_add(self, inst)

    def _drain_and_barrier(self, tick_clock, wait_clock):
        nc = self.nc
        drain_bi = nc.sync.drain()
        wait_clock.add_sem_waits(
            drain_bi.ins, ScopedClock({None: tick_clock.global_clock})
        )
        si = drain_bi.ins.sync_info
        if si is not None and si.on_wait and len(si.on_wait) > 1:
            # Distribute the extra waits across all engines' single-wait NOPs
            # so they resolve in parallel (this walrus allows only one wait
            # per instruction); the following all_engine_barrier makes the
            # union globally visible.
            waits = list(si.on_wait)
            si.on_wait = waits[:1]
            wait_engines = [nc.scalar, nc.gpsimd, nc.vector, nc.tensor, nc.sync]
            for j, w in enumerate(waits[1:]):
                nop_bi = wait_engines[j % len(wait_engines)].nop(
                    nofuse=True, hint="wait_split"
                )
                nop_bi.ins.sync_info = mybir.SyncInfo(on_wait=[w], on_update=[])

        nc.all_engine_barrier()
        assert self.sems is not None
        popped = nc._tile_sem_poison_stack.pop()
        assert popped is self._sem_poison
        # Inline clear_and_free_semaphores, but reset the sems with batched
        # multi-update NOPs (sem-wr-imm 0) instead of the
        # EVENT_SEMAPHORE_RANGE_CLEAR InstISA this walrus can't ingest
        # ("ISA wrong length"). Only *waits* are limited to one per
        # instruction; updates can be batched.
        from concourse.bass import compact_to_ranges

        sems = list(self.sems.allocated().values())
        if sems:
            sem_nums = [s.num for s in sems]
            for sem_range in compact_to_ranges(sem_nums):
                assert nc._state.free_isdisjoint(sem_range)
                nc.gpsimd.dma_reset(sem_range)
            reset_engines = [nc.gpsimd, nc.scalar, nc.vector, nc.tensor, nc.sync]
            for j, s in enumerate(sems):
                nop_bi = reset_engines[j % len(reset_engines)].nop(
                    nofuse=True, hint="sem_reset"
                )
                nop_bi.ins.sync_info = mybir.SyncInfo(
                    on_wait=[],
                    on_update=[
                        mybir.SyncUpdate(
                            sync_type="semaphore",
                            id=s.num,
                            ant_name=s.name,
                            update_mode="sem-wr-imm",
                            update_value=0,
                        )
                    ],
                )
            nc._state.prepend_free_semaphores(sem_nums)
            for poison_set in nc._tile_sem_poison_stack:
                poison_set.update(sem_nums)
        # No trailing all_engine_barrier: nothing follows the resets in this
        # standalone NEFF — each engine halts after its own reset NOP and the
        # runtime's completion detection waits for every engine anyway, so
        # the barrier only added a final sem round-trip to the measured span.
        # (The barrier BEFORE the resets stays: a sem must not be reset while
        # another engine could still be waiting on it.)

    tile_mod.TileContext._add_instruction = _add_instruction
    tile_mod.TileContext._drain_and_barrier = _drain_and_barrier
    _PATCHED = True


def _build_nc(L, H, K, n_tiles, repeat=1, w_mode="dma"):
    import concourse.bass as bass
    import concourse.mybir as mybir
    import concourse.tile as tile

    _patch_tile_drain()

    f32 = mybir.dt.float32
    i32 = mybir.dt.int32

    nc = bass.Bass("TRN2")
    seq = nc.dram_tensor("seq", [L, H], f32, kind="ExternalInput")
    # idx[p, 2*t] = src index for token t*P+p; idx[p, 2*t+1] = tgt index
    idx = nc.dram_tensor("idx", [P, 2 * n_tiles], i32, kind="ExternalInput")
    # [bias | w_src (H) | w_tgt (H)] as a single replicable strip
    wsb = nc.dram_tensor("wsb", [1, 2 * H + 1], f32, kind="ExternalInput")
    if w_mode == "wcat":
        wcat = nc.dram_tensor("wcat", [P, 2 * H + 1], f32, kind="ExternalInput")
    out = nc.dram_tensor("out", [K, K], f32, kind="ExternalOutput")

    last_t = n_tiles - 1
    Hs = (3 * H) // 4  # uneven split of the last tgt gather: 3/4 then 1/4
    mult = mybir.AluOpType.mult
    add = mybir.AluOpType.add

    with tile.TileContext(nc) as tc:
        with (
            tc.tile_pool(name="cpool", bufs=1) as cpool,
            tc.tile_pool(name="wpool", bufs=2) as wpool,
        ):
            idx_t = cpool.tile([P, 2 * n_tiles], i32)
            nc.sync.dma_start(out=idx_t[:], in_=idx[:])
            ws = cpool.tile([1, 2 * H + 1], f32)
            warm = cpool.tile([1, 1], f32)
            if w_mode == "dma":
                nc.scalar.dma_start(out=ws[:], in_=wsb[:])
                # warm the ACT Identity table during the idle head so the
                # per-tile column broadcasts (offloaded to ACT) don't pay the
                # ~1.3 us one-time table load mid-stream
                nc.scalar.activation(
                    out=warm[0:1, 0:1],
                    in_=ws[0:1, 0:1],
                    func=mybir.ActivationFunctionType.Identity,
                    scale=0.0,
                )

            def gather(dst, idx_ap, element_offset=0):
                nc.gpsimd.indirect_dma_start(
                    out=dst,
                    out_offset=None,
                    in_=seq[:],
                    in_offset=bass.IndirectOffsetOnAxis(ap=idx_ap, axis=0),
                    element_offset=element_offset,
                )

            def mul_reduce(prod_ap, gath_ap, w_ap, d_ap):
                # single DVE op: prod = gath * w, d = sum(prod)
                nc.vector.scalar_tensor_tensor(
                    out=prod_ap,
                    in0=gath_ap,
                    scalar=1.0,
                    in1=w_ap,
                    op0=mult,
                    op1=mult,
                    accum_out=d_ap,
                )

            def one_rep():
                # --- weights: 8 KiB strip + on-chip partition replication ---
                wb = wpool.tile([P, 2 * H + 1], f32, tag="wb")
                if w_mode == "dma":
                    # chunk 1 = bias + w_src (unblocks the first mul_reduce),
                    # chunk 2 = w_tgt; SBUF->SBUF on the ACT HWDGE ring with a
                    # 0-stride repeat dim: no HBM traffic, overlaps gathers.
                    c1 = 1 + H
                    nc.scalar.dma_start(
                        out=wb[:, 0:c1],
                        in_=ws[0:1, 0:c1].unsqueeze(1).broadcast_to((1, P, c1)),
                    )
                    nc.scalar.dma_start(
                        out=wb[:, c1 : 2 * H + 1],
                        in_=ws[0:1, c1 : 2 * H + 1]
                        .unsqueeze(1)
                        .broadcast_to((1, P, H)),
                    )
                else:  # host-replicated fallback: 1 MiB HBM read
                    nc.scalar.dma_start(out=wb[:], in_=wcat[:])
                b_ap = wb[:, 0:1]

                def w_src(rr):
                    return wb[rr, 1 : 1 + H]

                def w_tgt_sl(rr, lo, hi):
                    return wb[rr, 1 + H + lo : 1 + H + hi]

                # --- gathers (issued eagerly, stores interleaved late) ---
                # per tile, src||tgt rows land side by side in one (P, 2H)
                # tile; fetched by separate DMAs (see issue_gathers)
                cst = [
                    wpool.tile([P, 2 * H], f32, tag=f"cst{t}", name=f"cst{t}")
                    for t in range(n_tiles)
                ]

                def issue_gathers(t):
                    # src and tgt rows fetched by separate DMAs: the src
                    # mul-reduce overlaps the tgt transfer, keeping DVE fed
                    # at the per-row cadence (a merged 2-row gather stalls
                    # each tile's compute for the full doubled transfer)
                    rows = min(P, K - t * P)
                    rr = slice(0, rows)
                    if t != last_t:
                        gather(cst[t][rr, 0:H], idx_t[rr, 2 * t : 2 * t + 1])
                        gather(
                            cst[t][rr, H : 2 * H],
                            idx_t[rr, 2 * t + 1 : 2 * t + 2],
                        )
                    else:
                        # tail: src full, tgt split 3/4+1/4 so only a short
                        # reduce remains after the last gather lands
                        gather(cst[t][rr, 0:H], idx_t[rr, 2 * t : 2 * t + 1])
                        idx3 = idx_t[rr, 2 * t + 1 : 2 * t + 2]
                        gather(cst[t][rr, H : H + Hs], idx3)
                        gather(cst[t][rr, H + Hs :], idx3, element_offset=Hs)

                ps = wpool.tile([P, H], f32, tag="ps")  # shared DVE scratch
                obts = []

                def compute_tile(t):
                    rows = min(P, K - t * P)
                    rr = slice(0, rows)
                    ds = wpool.tile([P, 1], f32, tag=f"ds{t}", name=f"ds{t}")
                    dt = wpool.tile([P, 1], f32, tag=f"dt{t}", name=f"dt{t}")
                    col = wpool.tile([P, 1], f32, tag=f"col{t}", name=f"col{t}")
                    obt = wpool.tile([P, K], f32, tag=f"obt{t}", name=f"obt{t}")
                    mul_reduce(ps[rr], cst[t][rr, 0:H], w_src(rr), ds[rr, 0:1])
                    if t != last_t:
                        mul_reduce(
                            ps[rr], cst[t][rr, H : 2 * H], w_tgt_sl(rr, 0, H), dt[rr, 0:1]
                        )
                        # col = (ds + bias) + dt in one DVE op
                        nc.vector.scalar_tensor_tensor(
                            out=col[rr, 0:1],
                            in0=ds[rr, 0:1],
                            scalar=b_ap[rr],
                            in1=dt[rr, 0:1],
                            op0=add,
                            op1=add,
                        )
                    else:
                        dt2 = wpool.tile([P, 1], f32, tag="dt2")
                        col0 = wpool.tile([P, 1], f32, tag="col0")
                        mul_reduce(
                            ps[rr, 0:Hs],
                            cst[t][rr, H : H + Hs],
                            w_tgt_sl(rr, 0, Hs),
                            dt[rr, 0:1],
                        )
                        nc.vector.scalar_tensor_tensor(
                            out=col0[rr, 0:1],
                            in0=ds[rr, 0:1],
                            scalar=b_ap[rr],
                            in1=dt[rr, 0:1],
                            op0=add,
                            op1=add,
                        )
                        mul_reduce(
                            ps[rr, Hs:H],
                            cst[t][rr, H + Hs : 2 * H],
                            w_tgt_sl(rr, Hs, H),
                            dt2[rr, 0:1],
                        )
                        nc.vector.tensor_tensor(
                            out=col[rr, 0:1],
                            in0=col0[rr, 0:1],
                            in1=dt2[rr, 0:1],
                            op=add,
                        )
                    # broadcast col over the K output columns (in * 0 + col).
                    # Mid-tiles run on the otherwise-idle ACT engine to keep
                    # DVE tracking the gather stream; the last tile stays on
                    # DVE to avoid a cross-engine hop on the kernel tail.
                    if t != last_t and w_mode == "dma":
                        nc.scalar.activation(
                            out=obt[rr],
                            in_=cst[t][rr, 0:K],
                            func=mybir.ActivationFunctionType.Identity,
                            scale=0.0,
                            bias=col[rr, 0:1],
                        )
                    else:
                        nc.vector.tensor_scalar(
                            out=obt[rr],
                            in0=cst[t][rr, 0:K],
                            scalar1=0.0,
                            scalar2=col[rr, 0:1],
                            op0=mult,
                            op1=add,
                        )
                    obts.append((t, rows, obt))

                def issue_store(t, rows, obt):
                    rr = slice(0, rows)
                    if t != last_t:
                        eng = nc.sync if t % 2 == 0 else nc.scalar
                        eng.dma_start(out=out[t * P : t * P + rows, :], in_=obt[rr])
                    else:
                        Kh = K // 2
                        nc.scalar.dma_start(
                            out=out[t * P : t * P + rows, 0:Kh], in_=obt[rr, 0:Kh]
                        )
                        nc.sync.dma_start(
                            out=out[t * P : t * P + rows, Kh:K], in_=obt[rr, Kh:K]
                        )

                # Gathers for the first tiles go out immediately; each later
                # tile's gathers are followed by the store of the tile whose
                # data is ready by then, keeping the DMA stream dense and the
                # final stores off the kernel tail.
                for t in range(min(2, n_tiles)):
                    issue_gathers(t)
                for t in range(n_tiles):
                    compute_tile(t)
                    if t + 2 < n_tiles:
                        issue_gathers(t + 2)
                    if t >= 1:
                        issue_store(*obts[t - 1])
                issue_store(*obts[last_t])

            for _ in range(repeat):
                one_rep()
    return nc


_CACHE = {}

# test.py can flip these to get a profiled run
TRACE = False
LAST_RESULTS = None


def prepare_in_maps(seq, src_mask, tgt_mask, w, b, K):
    """Host-side metadata prep: mask -> ordered gather indices (matches the
    reference's stable argsort semantics exactly) + the [bias|w] strip."""
    B, L, H = seq.shape
    n_tiles = math.ceil(K / P)

    src_idx = np.argsort(~src_mask, axis=1, kind="stable")[:, :K].astype(np.int32)
    tgt_idx = np.argsort(~tgt_mask, axis=1, kind="stable")[:, :K].astype(np.int32)

    # Per-core index layout (P, 2*n_tiles): interleave src/tgt per tile
    idx_host = np.zeros((B, P, 2 * n_tiles), dtype=np.int32)
    for t in range(n_tiles):
        rows = min(P, K - t * P)
        idx_host[:, :rows, 2 * t] = src_idx[:, t * P : t * P + rows]
        idx_host[:, :rows, 2 * t + 1] = tgt_idx[:, t * P : t * P + rows]

    wsb_host = np.concatenate(
        [np.asarray(b[:1], np.float32), np.asarray(w, np.float32)]
    )[None, :]
    wsb_host = np.ascontiguousarray(wsb_host)
    # host-replicated variant, only consumed by w_mode="wcat" builds (A/B)
    wcat_host = np.ascontiguousarray(np.broadcast_to(wsb_host, (P, 2 * H + 1)))
    return [
        {
            "seq": seq[bb],
            "idx": idx_host[bb],
            "wsb": wsb_host,
            "wcat": wcat_host,
        }
        for bb in range(B)
    ]


def kernel(sequence_output, source_mask, target_mask, weight, bias, num_tokens):
    global LAST_RESULTS
    from concourse.bass_utils import run_bass_kernel_spmd

    seq = np.ascontiguousarray(np.asarray(sequence_output, dtype=np.float32))
    src_mask = np.asarray(source_mask, dtype=bool)
    tgt_mask = np.asarray(target_mask, dtype=bool)
    w = np.asarray(weight, dtype=np.float32)
    b = np.asarray(bias, dtype=np.float32)
    K = int(num_tokens)

    B, L, H = seq.shape
    n_tiles = math.ceil(K / P)

    in_maps = prepare_in_maps(seq, src_mask, tgt_mask, w, b, K)

    key = (L, H, K, n_tiles)
    if key not in _CACHE:
        _CACHE[key] = _build_nc(L, H, K, n_tiles)
    nc = _CACHE[key]

    # one sample per core; batches beyond 8 run in chunks of 8 cores
    N_CORES = 8
    outs = []
    for lo in range(0, B, N_CORES):
        chunk = in_maps[lo : lo + N_CORES]
        res = run_bass_kernel_spmd(
            nc, chunk, core_ids=list(range(len(chunk))), trace=TRACE
        )
        LAST_RESULTS = res
        outs.extend(res.results[i]["out"] for i in range(len(chunk)))
    return np.stack(outs, axis=0)


# revision 33
# speedup vs baseline: 1.0690x; 1.0106x over previous
"""Trainium2 kernel for nn_BinaryTokenClassificationModel.

Math (per batch sample b):
    src = seq[src_idx]           # (K, H) gather of K masked rows
    tgt = seq[tgt_idx]           # (K, H)
    col[s] = src[s] @ w[:H] + tgt[s] @ w[H:] + bias
    out[s, t] = col[s]           # broadcast over t

Sharding: data-parallel over batch B=8 across 8 NeuronCores (one sample per
core). Masks are converted to gather indices on the host (cheap O(B*L) argsort
metadata prep, matching the reference's stable-argsort semantics); the heavy
data stays on device.

Per core the kernel touches only the 2*K masked rows (4 MiB of the 16 MiB
sample) via SWDGE indirect-DMA row gathers plus the 1 MiB output write — the
memory roofline for this problem (~5.25 MiB of HBM traffic at ~360 GB/s/core):

  - weights+bias are loaded once as a [1, 2H+1] 8 KiB strip and replicated
    across the 128 SBUF partitions with an SBUF->SBUF HWDGE DMA whose source
    AP has a 0-stride repeat dim — no PE/PSUM round trip and, unlike a
    host-replicated (P, 2H) tensor, no extra 1 MiB of HBM reads (the sim's
    single-FIFO DMA device charges it like HBM traffic, but on HW it rides
    the SBUF fabric (435 GB/s) inside the SDMA engines' headroom while the
    HBM pipe (~358 GB/s) stays gather-bound). The strip is laid out
    [bias | w_src | w_tgt] and broadcast in two chunks so the first
    mul-reduce only waits for the first chunk.
  - all 9 row gathers (4 tiles x {src,tgt} into one (128, 2H) tile per tile;
    the last tile's tgt split 3/4+1/4) are issued up-front right after the
    tiny index load, so the SWDGE descriptor ring keeps the SDMA engines
    saturated end-to-end. src/tgt stay SEPARATE DMAs: the src mul-reduce
    overlaps the tgt transfer (a merged 2-row gather stalls each tile's
    compute for the doubled transfer), which is worth more than the ~1 us
    fixed Q7 descriptor-generation cost the extra instruction costs.
  - per 128-token tile: ONE fused DVE scalar_tensor_tensor per mask computes
    the elementwise product AND its free-dim sum (accum_out) in a single 1x
    pass; a tiny STT folds the bias into col = (ds + b) + dt; a 2x
    tensor_scalar broadcasts col over t; HWDGE writes the (128, K) output
    slab, alternating the SP/ACT rings per tile, with stores issued between
    late gathers to keep them off the kernel tail.
  - tail: the last tile's tgt gather is split 768+256 elements so only a
    ~330 ns reduce remains after the final gather lands, and the last store
    is split across both HWDGE rings.
  - drain: the Tile-exit sem waits and sem resets are distributed across all
    five engines so the epilogue resolves in parallel.

Container quirks handled by _patch_tile_drain(): this walrus accepts at most
one sync wait per instruction (extra waits are split onto single-wait NOPs)
and cannot ingest EVENT_SEMAPHORE_RANGE_CLEAR (semaphores are reset via
per-sem sem-wr-imm NOP updates instead).
"""

import math

import numpy as np

P = 128  # SBUF partitions

_PATCHED = False


def _patch_tile_drain():
    """This container's walrus/bass accept only ONE sync wait per instruction,
    but Tile attaches one wait per outstanding dependency to a single
    instruction ("Too many sync wait commands"). Split the extra waits across
    dedicated single-wait NOPs inserted just before on the same engine (the
    engine executes in order, so waiting sequentially is equivalent)."""
    global _PATCHED
    if _PATCHED:
        return
    import concourse.mybir as mybir
    import concourse.tile as tile_mod
    from concourse.vector_clock import ScopedClock

    _orig_add = tile_mod.TileContext._add_instruction

    def _add_instruction(self, inst):
        si = inst.sync_info
        if si is not None and si.on_wait and len(si.on_wait) > 1:
            waits = list(si.on_wait)
            si.on_wait = waits[-1:]
            for j, w in enumerate(waits[:-1]):
                nop = mybir.InstNoOp(name=f"{inst.name}_ws{j}")
                nop.engine = inst.engine
                nop.sync_info = mybir.SyncInfo(on_wait=[w], on_update=[])
                _orig_add(self, nop)
        _orig_add(self, inst)

    def _drain_and_barrier(self, tick_clock, wait_clock):
        nc = self.nc
        drain_bi = nc.sync.drain()
        wait_clock.add_sem_waits(
            drain_bi.ins, ScopedClock({None: tick_clock.global_clock})
        )
        si = drain_bi.ins.sync_info
        if si is not None and si.on_wait and len(si.on_wait) > 1:
            # Distribute the extra waits across all engines' single-wait NOPs
            # so they resolve in parallel (this walrus allows only one wait
            # per instruction); the following all_engine_barrier makes the
            # union globally visible.
            waits = list(si.on_wait)
            si.on_wait = waits[:1]
            wait_engines = [nc.scalar, nc.gpsimd, nc.vector, nc.tensor, nc.sync]
            for j, w in enumerate(waits[1:]):
                nop_bi = wait_engines[j % len(wait_engines)].nop(
                    nofuse=True, hint="wait_split"
                )
                nop_bi.ins.sync_info = mybir.SyncInfo(on_wait=[w], on_update=[])

        nc.all_engine_barrier()
        assert self.sems is not None
        popped = nc._tile_sem_poison_stack.pop()
        assert popped is self._sem_poison
        # Inline clear_and_free_semaphores, but reset the sems with batched
        # multi-update NOPs (sem-wr-imm 0) instead of the
        # EVENT_SEMAPHORE_RANGE_CLEAR InstISA this walrus can't ingest
        # ("ISA wrong length"). Only *waits* are limited to one per
        # instruction; updates can be batched.
        from concourse.bass import compact_to_ranges

        sems = list(self.sems.allocated().values())
        if sems:
            sem_nums = [s.num for s in sems]
            for sem_range in compact_to_ranges(sem_nums):
                assert nc._state.free_isdisjoint(sem_range)
                nc.gpsimd.dma_reset(sem_range)
            reset_engines = [nc.gpsimd, nc.scalar, nc.vector, nc.tensor, nc.sync]
            for j, s in enumerate(sems):
                nop_bi = reset_engines[j % len(reset_engines)].nop(
                    nofuse=True, hint="sem_reset"
                )
                nop_bi.ins.sync_info = mybir.SyncInfo(
                    on_wait=[],
                    on_update=[
                        mybir.SyncUpdate(
                            sync_type="semaphore",
                            id=s.num,
                            ant_name=s.name,
                            update_mode="sem-wr-imm",
                            update_value=0,
                        )
                    ],
                )
            nc._state.prepend_free_semaphores(sem_nums)
            for poison_set in nc._tile_sem_poison_stack:
                poison_set.update(sem_nums)
        # No trailing all_engine_barrier: nothing follows the resets in this
        # standalone NEFF — each engine halts after its own reset NOP and the
        # runtime's completion detection waits for every engine anyway, so
        # the barrier only added a final sem round-trip to the measured span.
        # (The barrier BEFORE the resets stays: a sem must not be reset while
        # another engine could still be waiting on it.)

    tile_mod.TileContext._add_instruction = _add_instruction
    tile_mod.TileContext._drain_and_barrier = _drain_and_barrier
    _PATCHED = True


def _build_nc(L, H, K, n_tiles, repeat=1, w_mode="dma"):
    import concourse.bass as bass
    import concourse.mybir as mybir
    import concourse.tile as tile

    _patch_tile_drain()

    f32 = mybir.dt.float32
    i32 = mybir.dt.int32

    nc = bass.Bass("TRN2")
    seq = nc.dram_tensor("seq", [L, H], f32, kind="ExternalInput")
    # idx[p, 2*t] = src index for token t*P+p; idx[p, 2*t+1] = tgt index
    idx = nc.dram_tensor("idx", [P, 2 * n_tiles], i32, kind="ExternalInput")
    # [bias | w_src (H) | w_tgt (H)] as a single replicable strip
    wsb = nc.dram_tensor("wsb", [1, 2 * H + 1], f32, kind="ExternalInput")
    if w_mode == "wcat":
        wcat = nc.dram_tensor("wcat", [P, 2 * H + 1], f32, kind="ExternalInput")
    out = nc.dram_tensor("out", [K, K], f32, kind="ExternalOutput")

    last_t = n_tiles - 1
    Hs = (3 * H) // 4  # uneven split of the last tgt gather: 3/4 then 1/4
    mult = mybir.AluOpType.mult
    add = mybir.AluOpType.add

    with tile.TileContext(nc) as tc:
        with (
            tc.tile_pool(name="cpool", bufs=1) as cpool,
            tc.tile_pool(name="wpool", bufs=2) as wpool,
        ):
            idx_t = cpool.tile([P, 2 * n_tiles], i32)
            nc.sync.dma_start(out=idx_t[:], in_=idx[:])
            ws = cpool.tile([1, 2 * H + 1], f32)
            warm = cpool.tile([1, 1], f32)
            if w_mode == "dma":
                nc.scalar.dma_start(out=ws[:], in_=wsb[:])
                # warm the ACT Identity table during the idle head so the
                # per-tile column broadcasts (offloaded to ACT) don't pay the
                # ~1.3 us one-time table load mid-stream
                nc.scalar.activation(
                    out=warm[0:1, 0:1],
                    in_=ws[0:1, 0:1],
                    func=mybir.ActivationFunctionType.Identity,
                    scale=0.0,
                )

            def gather(dst, idx_ap, element_offset=0):
                nc.gpsimd.indirect_dma_start(
                    out=dst,
                    out_offset=None,
                    in_=seq[:],
                    in_offset=bass.IndirectOffsetOnAxis(ap=idx_ap, axis=0),
                    element_offset=element_offset,
                )

            def mul_reduce(prod_ap, gath_ap, w_ap, d_ap):
                # single DVE op: prod = gath * w, d = sum(prod)
                nc.vector.scalar_tensor_tensor(
                    out=prod_ap,
                    in0=gath_ap,
                    scalar=1.0,
                    in1=w_ap,
                    op0=mult,
                    op1=mult,
                    accum_out=d_ap,
                )

            def one_rep():
                # --- weights: 8 KiB strip + on-chip partition replication ---
                wb = wpool.tile([P, 2 * H + 1], f32, tag="wb")
                if w_mode == "dma":
                    # chunk 1 = bias + w_src (unblocks the first mul_reduce),
                    # chunk 2 = w_tgt; SBUF->SBUF on the ACT HWDGE ring with a
                    # 0-stride repeat dim: no HBM traffic, overlaps gathers.
                    c1 = 1 + H
                    nc.scalar.dma_start(
                        out=wb[:, 0:c1],
                        in_=ws[0:1, 0:c1].unsqueeze(1).broadcast_to((1, P, c1)),
                    )
                    nc.scalar.dma_start(
                        out=wb[:, c1 : 2 * H + 1],
                        in_=ws[0:1, c1 : 2 * H + 1]
                        .unsqueeze(1)
                        .broadcast_to((1, P, H)),
                    )
                else:  # host-replicated fallback: 1 MiB HBM read
                    nc.scalar.dma_start(out=wb[:], in_=wcat[:])
                b_ap = wb[:, 0:1]

                def w_src(rr):
                    return wb[rr, 1 : 1 + H]

                def w_tgt_sl(rr, lo, hi):
                    return wb[rr, 1 + H + lo : 1 + H + hi]

                # --- gathers (issued eagerly, stores interleaved late) ---
                # per tile, src||tgt rows land side by side in one (P, 2H)
                # tile; fetched by separate DMAs (see issue_gathers)
                cst = [
                    wpool.tile([P, 2 * H], f32, tag=f"cst{t}", name=f"cst{t}")
                    for t in range(n_tiles)
                ]

                def issue_gathers(t):
                    # src and tgt rows fetched by separate DMAs: the src
                    # mul-reduce overlaps the tgt transfer, keeping DVE fed
                    # at the per-row cadence (a merged 2-row gather stalls
                    # each tile's compute for the full doubled transfer)
                    rows = min(P, K - t * P)
                    rr = slice(0, rows)
                    if t != last_t:
                        gather(cst[t][rr, 0:H], idx_t[rr, 2 * t : 2 * t + 1])
                        gather(
                            cst[t][rr, H : 2 * H],
                            idx_t[rr, 2 * t + 1 : 2 * t + 2],
                        )
                    else:
                        # tail: BOTH rows split 3/4+1/4 so the DVE reduces
                        # drain incrementally and only a short chain remains
                        # after the final gather chunk lands
                        idx3s = idx_t[rr, 2 * t : 2 * t + 1]
                        idx3t = idx_t[rr, 2 * t + 1 : 2 * t + 2]
                        gather(cst[t][rr, 0:Hs], idx3s)
                        gather(cst[t][rr, Hs:H], idx3s, element_offset=Hs)
                        gather(cst[t][rr, H : H + Hs], idx3t)
                        gather(cst[t][rr, H + Hs :], idx3t, element_offset=Hs)

                ps = wpool.tile([P, H], f32, tag="ps")  # shared DVE scratch
                obts = []

                def compute_tile(t):
                    rows = min(P, K - t * P)
                    rr = slice(0, rows)
                    ds = wpool.tile([P, 1], f32, tag=f"ds{t}", name=f"ds{t}")
                    dt = wpool.tile([P, 1], f32, tag=f"dt{t}", name=f"dt{t}")
                    col = wpool.tile([P, 1], f32, tag=f"col{t}", name=f"col{t}")
                    obt = wpool.tile([P, K], f32, tag=f"obt{t}", name=f"obt{t}")
                    mul_reduce(ps[rr], cst[t][rr, 0:H], w_src(rr), ds[rr, 0:1])
                    if t != last_t:
                        mul_reduce(
                            ps[rr], cst[t][rr, H : 2 * H], w_tgt_sl(rr, 0, H), dt[rr, 0:1]
                        )
                        # col = (ds + bias) + dt in one DVE op
                        nc.vector.scalar_tensor_tensor(
                            out=col[rr, 0:1],
                            in0=ds[rr, 0:1],
                            scalar=b_ap[rr],
                            in1=dt[rr, 0:1],
                            op0=add,
                            op1=add,
                        )
                    else:
                        dt2 = wpool.tile([P, 1], f32, tag="dt2")
                        col0 = wpool.tile([P, 1], f32, tag="col0")
                        mul_reduce(
                            ps[rr, 0:Hs],
                            cst[t][rr, H : H + Hs],
                            w_tgt_sl(rr, 0, Hs),
                            dt[rr, 0:1],
                        )
                        nc.vector.scalar_tensor_tensor(
                            out=col0[rr, 0:1],
                            in0=ds[rr, 0:1],
                            scalar=b_ap[rr],
                            in1=dt[rr, 0:1],
                            op0=add,
                            op1=add,
                        )
                        mul_reduce(
                            ps[rr, Hs:H],
                            cst[t][rr, H + Hs : 2 * H],
                            w_tgt_sl(rr, Hs, H),
                            dt2[rr, 0:1],
                        )
                        nc.vector.tensor_tensor(
                            out=col[rr, 0:1],
                            in0=col0[rr, 0:1],
                            in1=dt2[rr, 0:1],
                            op=add,
                        )
                    # broadcast col over the K output columns (in * 0 + col).
                    # Mid-tiles run on the otherwise-idle ACT engine to keep
                    # DVE tracking the gather stream; the last tile stays on
                    # DVE to avoid a cross-engine hop on the kernel tail.
                    if t != last_t and w_mode == "dma":
                        nc.scalar.activation(
                            out=obt[rr],
                            in_=cst[t][rr, 0:K],
                            func=mybir.ActivationFunctionType.Identity,
                            scale=0.0,
                            bias=col[rr, 0:1],
                        )
                    else:
                        nc.vector.tensor_scalar(
                            out=obt[rr],
                            in0=cst[t][rr, 0:K],
                            scalar1=0.0,
                            scalar2=col[rr, 0:1],
                            op0=mult,
                            op1=add,
                        )
                    obts.append((t, rows, obt))

                def issue_store(t, rows, obt):
                    rr = slice(0, rows)
                    if t != last_t:
                        eng = nc.sync if t % 2 == 0 else nc.scalar
                        eng.dma_start(out=out[t * P : t * P + rows, :], in_=obt[rr])
                    else:
                        Kh = K // 2
                        nc.scalar.dma_start(
                            out=out[t * P : t * P + rows, 0:Kh], in_=obt[rr, 0:Kh]
                        )
                        nc.sync.dma_start(
                            out=out[t * P : t * P + rows, Kh:K], in_=obt[rr, Kh:K]
                        )

                # Gathers for the first tiles go out immediately; each later
                # tile's gathers are followed by the store of the tile whose
                # data is ready by then, keeping the DMA stream dense and the
                # final stores off the kernel tail.
                for t in range(min(2, n_tiles)):
                    issue_gathers(t)
                for t in range(n_tiles):
                    compute_tile(t)
                    if t + 2 < n_tiles:
                        issue_gathers(t + 2)
                    if t >= 1:
                        issue_store(*obts[t - 1])
                issue_store(*obts[last_t])

            for _ in range(repeat):
                one_rep()
    return nc


_CACHE = {}

# test.py can flip these to get a profiled run
TRACE = False
LAST_RESULTS = None


def prepare_in_maps(seq, src_mask, tgt_mask, w, b, K):
    """Host-side metadata prep: mask -> ordered gather indices (matches the
    reference's stable argsort semantics exactly) + the [bias|w] strip."""
    B, L, H = seq.shape
    n_tiles = math.ceil(K / P)

    src_idx = np.argsort(~src_mask, axis=1, kind="stable")[:, :K].astype(np.int32)
    tgt_idx = np.argsort(~tgt_mask, axis=1, kind="stable")[:, :K].astype(np.int32)

    # Per-core index layout (P, 2*n_tiles): interleave src/tgt per tile
    idx_host = np.zeros((B, P, 2 * n_tiles), dtype=np.int32)
    for t in range(n_tiles):
        rows = min(P, K - t * P)
        idx_host[:, :rows, 2 * t] = src_idx[:, t * P : t * P + rows]
        idx_host[:, :rows, 2 * t + 1] = tgt_idx[:, t * P : t * P + rows]

    wsb_host = np.concatenate(
        [np.asarray(b[:1], np.float32), np.asarray(w, np.float32)]
    )[None, :]
    wsb_host = np.ascontiguousarray(wsb_host)
    # host-replicated variant, only consumed by w_mode="wcat" builds (A/B)
    wcat_host = np.ascontiguousarray(np.broadcast_to(wsb_host, (P, 2 * H + 1)))
    return [
        {
            "seq": seq[bb],
            "idx": idx_host[bb],
            "wsb": wsb_host,
            "wcat": wcat_host,
        }
        for bb in range(B)
    ]


def kernel(sequence_output, source_mask, target_mask, weight, bias, num_tokens):
    global LAST_RESULTS
    from concourse.bass_utils import run_bass_kernel_spmd

    seq = np.ascontiguousarray(np.asarray(sequence_output, dtype=np.float32))
    src_mask = np.asarray(source_mask, dtype=bool)
    tgt_mask = np.asarray(target_mask, dtype=bool)
    w = np.asarray(weight, dtype=np.float32)
    b = np.asarray(bias, dtype=np.float32)
    K = int(num_tokens)

    B, L, H = seq.shape
    n_tiles = math.ceil(K / P)

    in_maps = prepare_in_maps(seq, src_mask, tgt_mask, w, b, K)

    key = (L, H, K, n_tiles)
    if key not in _CACHE:
        _CACHE[key] = _build_nc(L, H, K, n_tiles)
    nc = _CACHE[key]

    # one sample per core; batches beyond 8 run in chunks of 8 cores
    N_CORES = 8
    outs = []
    for lo in range(0, B, N_CORES):
        chunk = in_maps[lo : lo + N_CORES]
        res = run_bass_kernel_spmd(
            nc, chunk, core_ids=list(range(len(chunk))), trace=TRACE
        )
        LAST_RESULTS = res
        outs.extend(res.results[i]["out"] for i in range(len(chunk)))
    return np.stack(outs, axis=0)
